# revision 1
# baseline (speedup 1.0000x reference)
"""MoE FFN (8 experts, top-2) on 8 Trainium2 NeuronCores.

Expert parallelism: the (tiny) router runs on host with the exact same jax
ops as the reference; tokens are dispatched to their top-2 experts; core e
runs expert e's FFN over its routed tokens (capacity-padded so all cores run
the same SPMD program); the host applies the combine weights and
scatter-adds the two expert outputs per token.

On-device layout: all matmul operands keep the contraction dim on SBUF
partitions. Weights are host-pre-transposed (w1t = w1[e].T contiguous,
w2t = w2[e].T contiguous, bf16) and stay resident in SBUF; activations live
as G.T = gelu(X W1.T).T in [H, tokens] layout so layer 2 consumes them
directly. PSUM accumulates in f32; layer-1 bias rides the gelu on ScalarE,
layer-2 bias is fused into the PSUM eviction on VectorE.
"""

import numpy as np
import ml_dtypes

N_EXPERTS = 8
TOP_K = 2
C = 1024
H = 4096
P = 128
T_TILE = 512
KO1 = C // P   # 8 contraction chunks for layer 1
KO2 = H // P   # 32 contraction chunks for layer 2

_nc_cache = {}


def _build_nc(cap: int, act: str = "gelu"):
    import concourse.mybir as mybir
    import concourse.tile as tile
    from concourse import bacc

    bf16 = mybir.dt.bfloat16
    f32 = mybir.dt.float32

    nc = bacc.Bacc()
    xt = nc.dram_tensor("xt", [C, cap], bf16, kind="ExternalInput")
    w1t = nc.dram_tensor("w1t", [C, H], bf16, kind="ExternalInput")
    w2t = nc.dram_tensor("w2t", [H, C], bf16, kind="ExternalInput")
    # biases come host-pre-swizzled: [P, H//P] / [P, C//P], partition-major
    b1 = nc.dram_tensor("b1", [P, KO2], f32, kind="ExternalInput")
    b2 = nc.dram_tensor("b2", [P, KO1], f32, kind="ExternalInput")
    yt = nc.dram_tensor("yt", [C, cap], f32, kind="ExternalOutput")

    xt_r = xt.rearrange("(ko ki) t -> ki ko t", ki=P)
    w1t_r = w1t.rearrange("(ko ki) h -> ki ko h", ki=P)
    w2t_r = w2t.rearrange("(ko ki) c -> ki ko c", ki=P)
    yt_r = yt.rearrange("(co p) t -> p co t", p=P)

    # Balanced token tiles: per-tile matmul count is fixed (512) whatever T
    # is, so equal splits amortize issue overhead best; tiles under ~256
    # tokens fall below the LDWEIGHTS floor (~100ns/MM) and waste PE time.
    n_tiles = -(-cap // T_TILE)
    k = cap // P
    tiles = [(k // n_tiles + (1 if i < k % n_tiles else 0)) * P for i in range(n_tiles)]
    assert sum(tiles) == cap and all(t <= T_TILE for t in tiles)

    gelu = {
        "gelu": mybir.ActivationFunctionType.Gelu_apprx_tanh,
        "gelu_erf": mybir.ActivationFunctionType.Gelu,
        "tanh": mybir.ActivationFunctionType.Tanh,
    }[act]

    with tile.TileContext(nc) as tc:
        with (
            tc.tile_pool(name="const", bufs=1) as const,
            tc.tile_pool(name="xp", bufs=2) as xp,
            tc.tile_pool(name="gp", bufs=1) as gp,
            tc.tile_pool(name="yp", bufs=4) as yp,
            tc.tile_pool(name="psum", bufs=8, space="PSUM") as psum,
        ):
            w1_sb = const.tile([P, KO1, H], bf16, tag="w1")
            w2_sb = const.tile([P, KO2, C], bf16, tag="w2")
            b1_sb = const.tile([P, KO2], f32, tag="b1")
            b2_sb = const.tile([P, KO1], f32, tag="b2")
            # The HWDGE stream drains serially in program order, so issue
            # loads in exactly the order the first layer-1 m-tiles consume
            # them: x(t0) ko-chunks interleaved with the first w1 h-quarter,
            # then biases, the rest of w1, then w2. Anything queued behind
            # the 16MB of weights would stall the first matmuls ~45us.
            x_tiles = {}
            x_tiles[0] = xp.tile([P, KO1, T_TILE], bf16, tag="x", name="x0")
            QW = 1024
            for ko in range(KO1):
                nc.sync.dma_start(
                    x_tiles[0][:, ko : ko + 1, : tiles[0]],
                    xt_r[:, ko : ko + 1, : tiles[0]],
                )
                nc.sync.dma_start(
                    w1_sb[:, ko : ko + 1, 0:QW], w1t_r[:, ko : ko + 1, 0:QW]
                )
            nc.sync.dma_start(b1_sb[:], b1[:])
            nc.sync.dma_start(b2_sb[:], b2[:])
            for q in range(QW, H, QW):
                for ko in range(KO1):
                    nc.sync.dma_start(
                        w1_sb[:, ko : ko + 1, q : q + QW],
                        w1t_r[:, ko : ko + 1, q : q + QW],
                    )
            for ko in range(KO2):
                nc.sync.dma_start(w2_sb[:, ko : ko + 1, :], w2t_r[:, ko : ko + 1, :])

            t0 = 0
            for ti, T in enumerate(tiles):
                # prefetch next x tile ahead of this tile's output DMAs
                if ti + 1 < len(tiles):
                    nt = tiles[ti + 1]
                    nt0 = t0 + T
                    x_tiles[ti + 1] = xp.tile(
                        [P, KO1, T_TILE], bf16, tag="x", name=f"x{ti + 1}"
                    )
                    nc.sync.dma_start(
                        x_tiles[ti + 1][:, :, :nt], xt_r[:, :, nt0 : nt0 + nt]
                    )
                x_sb = x_tiles.pop(ti)
                g_sb = gp.tile([P, KO2, T_TILE], bf16, tag="g")
                for m in range(KO2):
                    ph = psum.tile([P, T_TILE], f32, tag="ps")
                    for ko in range(KO1):
                        nc.tensor.matmul(
                            ph[:, :T],
                            w1_sb[:, ko, m * P : (m + 1) * P],
                            x_sb[:, ko, :T],
                            start=(ko == 0),
                            stop=(ko == KO1 - 1),
                        )
                    nc.scalar.activation(
                        g_sb[:, m, :T], ph[:, :T], gelu, bias=b1_sb[:, m : m + 1]
                    )
                for co in range(KO1):
                    py = psum.tile([P, T_TILE], f32, tag="ps")
                    for ho in range(KO2):
                        nc.tensor.matmul(
                            py[:, :T],
                            w2_sb[:, ho, co * P : (co + 1) * P],
                            g_sb[:, ho, :T],
                            start=(ho == 0),
                            stop=(ho == KO2 - 1),
                        )
                    y_sb = yp.tile([P, T_TILE], f32, tag="y")
                    nc.vector.tensor_scalar_add(
                        y_sb[:, :T], py[:, :T], b2_sb[:, co : co + 1]
                    )
                    nc.sync.dma_start(yt_r[:, co, t0 : t0 + T], y_sb[:, :T])
                t0 += T
    nc.finalize()
    return nc


def _route(flat_f32: np.ndarray, gate_w: np.ndarray):
    """Router, bit-matching the reference's jax ops (same env/backend)."""
    import jax
    import jax.numpy as jnp

    logits = jnp.asarray(flat_f32) @ jnp.asarray(gate_w).T
    probs = jax.nn.softmax(logits, axis=-1)
    top_p, top_i = jax.lax.top_k(probs, TOP_K)
    weights = top_p / (jnp.sum(top_p, axis=-1, keepdims=True) + 1e-8)
    return np.asarray(top_i), np.asarray(weights)


# results of the last device run, for test harness introspection
last_result = None


def _ensure_ntff_hook():
    """bass_utils' trace path imports antenv.axon_hooks, which the agent
    image's antenv lacks. Build the hook from trn_agent_boot's ctypes
    shim and inject a stand-in module."""
    import sys
    import types

    if "antenv.axon_hooks" in sys.modules:
        return
    try:
        from trn_agent_boot.trn_boot import _ntff_profile_via_ctypes

        hook = _ntff_profile_via_ctypes("/opt/axon/libaxon_pjrt.so")
    except Exception:
        hook = None
    m = types.ModuleType("antenv.axon_hooks")
    m.get_axon_ntff_profile_hook = lambda: hook
    m.set_axon_ntff_profile_hook = lambda h: None
    sys.modules["antenv.axon_hooks"] = m


def kernel(x, gate_w, w1, b1, w2, b2):
    from concourse.bass_utils import run_bass_kernel_spmd

    x = np.asarray(x)
    B, N, _ = x.shape
    flat = np.ascontiguousarray(x.reshape(-1, C), dtype=np.float32)
    T = flat.shape[0]

    top_i, weights = _route(flat, np.asarray(gate_w, dtype=np.float32))

    # token ids and combine weights per expert
    idx_e = []
    g_e = []
    for e in range(N_EXPERTS):
        rows, cols = np.nonzero(top_i == e)
        idx_e.append(rows.astype(np.int64))
        g_e.append(weights[rows, cols].astype(np.float32))
    counts = np.array([len(i) for i in idx_e])
    cap = max(int(-(-counts.max() // P) * P), P)

    nc = _nc_cache.get(cap)
    if nc is None:
        nc = _build_nc(cap)
        _nc_cache[cap] = nc

    bf16 = ml_dtypes.bfloat16
    w1_t = np.ascontiguousarray(np.asarray(w1).transpose(0, 2, 1)).astype(bf16)
    w2_t = np.ascontiguousarray(np.asarray(w2).transpose(0, 2, 1)).astype(bf16)
    # pre-swizzle biases to [P, n_chunks]: partition p of chunk m holds b[m*P+p]
    b1_f = np.ascontiguousarray(
        np.asarray(b1, dtype=np.float32).reshape(N_EXPERTS, KO2, P).transpose(0, 2, 1)
    )
    b2_f = np.ascontiguousarray(
        np.asarray(b2, dtype=np.float32).reshape(N_EXPERTS, KO1, P).transpose(0, 2, 1)
    )

    in_maps = []
    for e in range(N_EXPERTS):
        xe = np.zeros((C, cap), dtype=bf16)
        xe[:, : counts[e]] = flat[idx_e[e]].T.astype(bf16)
        in_maps.append(
            {
                "xt": xe,
                "w1t": w1_t[e],
                "w2t": w2_t[e],
                "b1": b1_f[e],
                "b2": b2_f[e],
            }
        )

    import os

    trace = bool(int(os.environ.get("MOE_TRACE", "0")))
    if trace:
        _ensure_ntff_hook()

    global last_result
    res = run_bass_kernel_spmd(
        nc,
        in_maps,
        core_ids=list(range(N_EXPERTS)),
        trace=trace,
    )
    last_result = res

    out = np.zeros((T, C), dtype=np.float32)
    for e in range(N_EXPERTS):
        ye = res.results[e]["yt"]  # [C, cap] f32
        cnt = counts[e]
        out[idx_e[e]] += g_e[e][:, None] * ye[:, :cnt].T
    return out.reshape(B, N, C)



# revision 2
# speedup vs baseline: 1.0029x; 1.0029x over previous
"""MoE FFN (8 experts, top-2) on 8 Trainium2 NeuronCores.

Expert parallelism with half-expert load balancing: the router runs on host
(same jax ops as the reference); each expert's FFN is split along the hidden
dim into two halves computed on two different cores, and each core serves one
half of a *large* expert (segment A) plus one half of a *small* expert
(segment B), pairing rank k with rank 7-k by token count. This caps per-core
work at (max_large + max_small)/2 token-equivalents instead of max over all
experts. The host sums the two half-expert partial outputs, adds b2, applies
the combine weights, and scatter-adds into the final output.

On-device layout: contraction dim lives on SBUF partitions for every matmul.
Weights are host-pre-transposed bf16 and stay resident in SBUF; activations
live as G.T = gelu(X W1h.T).T in [H/2, tokens] layout so layer 2 consumes
them directly. PSUM accumulates f32; the layer-1 bias rides the gelu on
ScalarE; layer-2 output is evicted to bf16 (b2 is added on host). Layer 2's
contraction is issued in two halves so the last-gelu latency hides under the
first half's matmuls instead of stalling the PE at each tile boundary.
"""

import numpy as np
import ml_dtypes

N_EXPERTS = 8
TOP_K = 2
C = 1024
H = 4096
HH = H // 2
P = 128
T_TILE = 512
KO1 = C // P   # 8 contraction chunks for layer 1
MH = HH // P   # 16 hidden chunks per half-expert

_nc_cache = {}


def _split_tiles(cap: int):
    # Balanced token tiles in P units; every tile <= T_TILE and >= 256ish so
    # matmuls stay above the LDWEIGHTS floor.
    n_tiles = -(-cap // T_TILE)
    k = cap // P
    tiles = [(k // n_tiles + (1 if i < k % n_tiles else 0)) * P for i in range(n_tiles)]
    assert sum(tiles) == cap and all(t <= T_TILE for t in tiles)
    return tiles


def _build_nc(capA: int, capB: int):
    import concourse.mybir as mybir
    import concourse.tile as tile
    from concourse import bacc

    bf16 = mybir.dt.bfloat16
    f32 = mybir.dt.float32
    gelu = mybir.ActivationFunctionType.Gelu_apprx_tanh

    nc = bacc.Bacc()
    dram = {}
    for s, cap in (("A", capA), ("B", capB)):
        dram[f"xt{s}"] = nc.dram_tensor(f"xt{s}", [C, cap], bf16, kind="ExternalInput")
        dram[f"w1t{s}"] = nc.dram_tensor(f"w1t{s}", [C, HH], bf16, kind="ExternalInput")
        dram[f"w2t{s}"] = nc.dram_tensor(f"w2t{s}", [HH, C], bf16, kind="ExternalInput")
        dram[f"b1{s}"] = nc.dram_tensor(f"b1{s}", [P, MH], f32, kind="ExternalInput")
        dram[f"yt{s}"] = nc.dram_tensor(f"yt{s}", [C, cap], bf16, kind="ExternalOutput")

    xr = {s: dram[f"xt{s}"].rearrange("(ko ki) t -> ki ko t", ki=P) for s in "AB"}
    w1r = {s: dram[f"w1t{s}"].rearrange("(ko ki) h -> ki ko h", ki=P) for s in "AB"}
    w2r = {s: dram[f"w2t{s}"].rearrange("(ho ki) c -> ki ho c", ki=P) for s in "AB"}
    yr = {s: dram[f"yt{s}"].rearrange("(co p) t -> p co t", p=P) for s in "AB"}

    tiles = {"A": _split_tiles(capA), "B": _split_tiles(capB)}
    # flat schedule across both segments: (seg, tile_idx_in_seg, T, t0)
    sched = []
    for s in "AB":
        t0 = 0
        for ti, T in enumerate(tiles[s]):
            sched.append((s, ti, T, t0))
            t0 += T

    with tile.TileContext(nc) as tc:
        with (
            tc.tile_pool(name="const", bufs=1) as const,
            tc.tile_pool(name="xp", bufs=2) as xp,
            tc.tile_pool(name="gp", bufs=1) as gp,
            tc.tile_pool(name="yp", bufs=4) as yp,
            tc.tile_pool(name="psum", bufs=8, space="PSUM") as psum,
        ):
            w1_sb = {
                s: const.tile([P, KO1, HH], bf16, tag=f"w1{s}", name=f"w1{s}")
                for s in "AB"
            }
            w2_sb = {
                s: const.tile([P, MH, C], bf16, tag=f"w2{s}", name=f"w2{s}")
                for s in "AB"
            }
            b1_sb = {
                s: const.tile([P, MH], f32, tag=f"b1{s}", name=f"b1{s}")
                for s in "AB"
            }

            # --- initial DMA queue: get the first matmuls running ASAP. ---
            # The HWDGE queue drains in program order, so issue exactly what
            # the first m-chunks consume first: x(tile0) interleaved with
            # w1A's m0 columns, then b1A, then w1A column ranges ordered by
            # first consumption, then w2A. Segment-B constants are issued
            # inside tile 1's body so xA1 stays ahead of them in the queue.
            x_tiles = {}
            T0 = tiles["A"][0]
            x_tiles[0] = xp.tile([P, KO1, T_TILE], bf16, tag="x", name="x0")
            for ko in range(KO1):
                nc.sync.dma_start(x_tiles[0][:, ko : ko + 1, :T0], xr["A"][:, ko : ko + 1, :T0])
                nc.sync.dma_start(w1_sb["A"][:, ko : ko + 1, 0:P], w1r["A"][:, ko : ko + 1, 0:P])
            nc.sync.dma_start(b1_sb["A"][:], dram["b1A"][:])
            for lo, hi in ((128, 256), (256, 512), (512, 1024), (1024, 2048)):
                for ko in range(KO1):
                    nc.sync.dma_start(
                        w1_sb["A"][:, ko : ko + 1, lo:hi], w1r["A"][:, ko : ko + 1, lo:hi]
                    )
            for ho in range(MH):
                nc.sync.dma_start(w2_sb["A"][:, ho : ho + 1, :], w2r["A"][:, ho : ho + 1, :])

            for gi, (s, ti, T, t0) in enumerate(sched):
                # prefetch next tile's x ahead of this tile's output DMAs
                if gi + 1 < len(sched):
                    ns, nti, nT, nt0 = sched[gi + 1]
                    x_tiles[gi + 1] = xp.tile(
                        [P, KO1, T_TILE], bf16, tag="x", name=f"x{gi + 1}"
                    )
                    nc.sync.dma_start(
                        x_tiles[gi + 1][:, :, :nT], xr[ns][:, :, nt0 : nt0 + nT]
                    )
                if gi == 1:
                    # segment-B constants: queued behind xA1, ahead of y(t1)+
                    nc.sync.dma_start(b1_sb["B"][:], dram["b1B"][:])
                    for lo, hi in ((0, 1024), (1024, 2048)):
                        for ko in range(KO1):
                            nc.sync.dma_start(
                                w1_sb["B"][:, ko : ko + 1, lo:hi],
                                w1r["B"][:, ko : ko + 1, lo:hi],
                            )
                    for ho in range(MH):
                        nc.sync.dma_start(
                            w2_sb["B"][:, ho : ho + 1, :], w2r["B"][:, ho : ho + 1, :]
                        )

                x_sb = x_tiles.pop(gi)
                g_sb = gp.tile([P, MH, T_TILE], bf16, tag="g", name=f"g{gi}")
                for m in range(MH):
                    ph = psum.tile([P, T_TILE], f32, tag="ps", name=f"ph{gi}_{m}")
                    for ko in range(KO1):
                        nc.tensor.matmul(
                            ph[:, :T],
                            w1_sb[s][:, ko, m * P : (m + 1) * P],
                            x_sb[:, ko, :T],
                            start=(ko == 0),
                            stop=(ko == KO1 - 1),
                        )
                    nc.scalar.activation(
                        g_sb[:, m, :T], ph[:, :T], gelu, bias=b1_sb[s][:, m : m + 1]
                    )
                # layer 2, contraction split in two: the first half only needs
                # g[:8], so it issues right behind the L1 matmuls while the
                # m=15 gelu drains; the second half lands long after.
                pys = [
                    psum.tile([P, T_TILE], f32, tag="ps", name=f"py{gi}_{co}")
                    for co in range(KO1)
                ]
                for co in range(KO1):
                    for ho in range(MH // 2):
                        nc.tensor.matmul(
                            pys[co][:, :T],
                            w2_sb[s][:, ho, co * P : (co + 1) * P],
                            g_sb[:, ho, :T],
                            start=(ho == 0),
                            stop=False,
                        )
                for co in range(KO1):
                    for ho in range(MH // 2, MH):
                        nc.tensor.matmul(
                            pys[co][:, :T],
                            w2_sb[s][:, ho, co * P : (co + 1) * P],
                            g_sb[:, ho, :T],
                            start=False,
                            stop=(ho == MH - 1),
                        )
                    y_sb = yp.tile([P, T_TILE], bf16, tag="y", name=f"y{gi}_{co}")
                    nc.vector.tensor_copy(y_sb[:, :T], pys[co][:, :T])
                    nc.sync.dma_start(yr[s][:, co, t0 : t0 + T], y_sb[:, :T])
    nc.finalize()
    return nc


def _route(flat_f32: np.ndarray, gate_w: np.ndarray):
    """Router, bit-matching the reference's jax ops (same env/backend)."""
    import jax
    import jax.numpy as jnp

    logits = jnp.asarray(flat_f32) @ jnp.asarray(gate_w).T
    probs = jax.nn.softmax(logits, axis=-1)
    top_p, top_i = jax.lax.top_k(probs, TOP_K)
    weights = top_p / (jnp.sum(top_p, axis=-1, keepdims=True) + 1e-8)
    return np.asarray(top_i), np.asarray(weights)


# results of the last device run, for test harness introspection
last_result = None


def _ensure_ntff_hook():
    """bass_utils' trace path imports antenv.axon_hooks, which the agent
    image's antenv lacks. Build the hook from trn_agent_boot's ctypes
    shim and inject a stand-in module."""
    import sys
    import types

    if "antenv.axon_hooks" in sys.modules:
        return
    try:
        from trn_agent_boot.trn_boot import _ntff_profile_via_ctypes

        hook = _ntff_profile_via_ctypes("/opt/axon/libaxon_pjrt.so")
    except Exception:
        hook = None
    m = types.ModuleType("antenv.axon_hooks")
    m.get_axon_ntff_profile_hook = lambda: hook
    m.set_axon_ntff_profile_hook = lambda h: None
    sys.modules["antenv.axon_hooks"] = m


def kernel(x, gate_w, w1, b1, w2, b2):
    from concourse.bass_utils import run_bass_kernel_spmd

    x = np.asarray(x)
    B, N, _ = x.shape
    flat = np.ascontiguousarray(x.reshape(-1, C), dtype=np.float32)
    w1 = np.asarray(w1, dtype=np.float32)
    w2 = np.asarray(w2, dtype=np.float32)
    b1 = np.asarray(b1, dtype=np.float32)
    b2 = np.asarray(b2, dtype=np.float32)

    top_i, weights = _route(flat, np.asarray(gate_w, dtype=np.float32))

    # token ids and combine weights per expert
    idx_e, g_e = [], []
    for e in range(N_EXPERTS):
        rows, cols = np.nonzero(top_i == e)
        idx_e.append(rows)
        g_e.append(weights[rows, cols].astype(np.float32))
    counts = np.array([len(i) for i in idx_e])

    # rank experts by load; segment A = big four, B = small four. Expert
    # ranked[r] runs as two hidden-halves on cores r and r+4; expert
    # ranked[7-r] likewise (segment B on the same core pair).
    ranked = np.argsort(-counts, kind="stable")
    pad = lambda n: max(int(-(-n // P) * P), P)
    capA = pad(int(counts[ranked[0]]))
    capB = pad(int(counts[ranked[4]]))

    key = (capA, capB)
    nc = _nc_cache.get(key)
    if nc is None:
        nc = _build_nc(capA, capB)
        _nc_cache[key] = nc

    bf16 = ml_dtypes.bfloat16

    # per-expert padded x (shared by the expert's two half-cores)
    xt = {}
    for s, cap, exps in (("A", capA, ranked[:4]), ("B", capB, ranked[4:])):
        for e in exps:
            xe = np.zeros((C, cap), dtype=bf16)
            xe[:, : counts[e]] = flat[idx_e[e]].T.astype(bf16)
            xt[int(e)] = xe

    in_maps = []
    for core in range(8):
        r, half = core % 4, core // 4
        m = {}
        for s, r_e in (("A", ranked[r]), ("B", ranked[7 - r])):
            e = int(r_e)
            lo, hi = half * HH, (half + 1) * HH
            m[f"xt{s}"] = xt[e]
            m[f"w1t{s}"] = np.ascontiguousarray(w1[e, lo:hi, :].T).astype(bf16)
            m[f"w2t{s}"] = np.ascontiguousarray(w2[e, :, lo:hi].T).astype(bf16)
            m[f"b1{s}"] = np.ascontiguousarray(
                b1[e, lo:hi].reshape(MH, P).T
            )
        in_maps.append(m)

    import os

    trace = bool(int(os.environ.get("MOE_TRACE", "0")))
    if trace:
        _ensure_ntff_hook()

    global last_result
    res = run_bass_kernel_spmd(
        nc,
        in_maps,
        core_ids=list(range(8)),
        trace=trace,
    )
    last_result = res

    out = np.zeros((flat.shape[0], C), dtype=np.float32)
    for r in range(4):
        for s, r_e in (("A", ranked[r]), ("B", ranked[7 - r])):
            e = int(r_e)
            cnt = counts[e]
            y = res.results[r][f"yt{s}"][:, :cnt].astype(np.float32)
            y += res.results[r + 4][f"yt{s}"][:, :cnt].astype(np.float32)
            out[idx_e[e]] += g_e[e][:, None] * (y.T + b2[e])
    return out.reshape(B, N, C)


# revision 4
# speedup vs baseline: 1.0057x; 1.0028x over previous
"""MoE FFN (8 experts, top-2) on 8 Trainium2 NeuronCores.

Expert parallelism with half-expert load balancing: the router runs on host
(same jax ops as the reference); each expert's FFN is split along the hidden
dim into two halves computed on two different cores, and each core serves one
half of a *large* expert (segment A) plus one half of a *small* expert
(segment B), pairing rank k with rank 7-k by token count. This caps per-core
work at (max_large + max_small)/2 token-equivalents instead of max over all
experts. The host sums the two half-expert partial outputs, adds b2, applies
the combine weights, and scatter-adds into the final output.

On-device layout: contraction dim lives on SBUF partitions for every matmul.
Weights are host-prearranged so each consumed [128,128] stationary block
arrives as part of a single [128 x 2KB-line] descriptor in exact consumption
order (m-major), and the initial loads are spread across three engine DMA
queues (sync/scalar/gpsimd) so the first matmul isn't gated on one queue's
cold-start ramp. PSUM accumulates f32; the layer-1 bias rides the gelu on
ScalarE; layer-2 output is evicted to bf16 (b2 is added on host). Layer 2's
contraction is issued in two halves so the last-gelu latency hides under the
first half's matmuls instead of stalling the PE at each tile boundary.
"""

import numpy as np
import ml_dtypes

N_EXPERTS = 8
TOP_K = 2
C = 1024
H = 4096
HH = H // 2
P = 128
T_TILE = 512
KO1 = C // P   # 8 contraction chunks for layer 1
MH = HH // P   # 16 hidden chunks per half-expert

_nc_cache = {}


def _split_tiles(cap: int, ascending: bool = False):
    # Balanced token tiles in P units; every tile <= T_TILE and big enough to
    # stay above the LDWEIGHTS floor.
    n_tiles = -(-cap // T_TILE)
    k = cap // P
    tiles = [(k // n_tiles + (1 if i < k % n_tiles else 0)) * P for i in range(n_tiles)]
    assert sum(tiles) == cap and all(t <= T_TILE for t in tiles)
    if ascending:
        tiles.sort()
    return tiles


def _build_nc(capA: int, capB: int):
    import concourse.mybir as mybir
    import concourse.tile as tile
    from concourse import bacc

    bf16 = mybir.dt.bfloat16
    f32 = mybir.dt.float32
    gelu = mybir.ActivationFunctionType.Gelu_apprx_tanh

    nc = bacc.Bacc()
    dram = {}
    for s, cap in (("A", capA), ("B", capB)):
        dram[f"xt{s}"] = nc.dram_tensor(f"xt{s}", [C, cap], bf16, kind="ExternalInput")
        # weights host-prearranged: [ki, m, ko*128+j] = w1[m*128+j, ko*128+ki]
        dram[f"w1t{s}"] = nc.dram_tensor(f"w1t{s}", [P, MH, C], bf16, kind="ExternalInput")
        dram[f"w2t{s}"] = nc.dram_tensor(f"w2t{s}", [P, MH, C], bf16, kind="ExternalInput")
        dram[f"b1{s}"] = nc.dram_tensor(f"b1{s}", [P, MH], f32, kind="ExternalInput")
        dram[f"yt{s}"] = nc.dram_tensor(f"yt{s}", [C, cap], bf16, kind="ExternalOutput")

    xr = {s: dram[f"xt{s}"].rearrange("(ko ki) t -> ki ko t", ki=P) for s in "AB"}
    yr = {s: dram[f"yt{s}"].rearrange("(co p) t -> p co t", p=P) for s in "AB"}

    # smallest tile first in A: less x to load before the first matmul while
    # the DMA engines are still ramping.
    tiles = {"A": _split_tiles(capA, ascending=True), "B": _split_tiles(capB)}
    sched = []
    for s in "AB":
        t0 = 0
        for ti, T in enumerate(tiles[s]):
            sched.append((s, ti, T, t0))
            t0 += T

    with tile.TileContext(nc) as tc:
        with (
            tc.tile_pool(name="const", bufs=1) as const,
            tc.tile_pool(name="xp", bufs=2) as xp,
            tc.tile_pool(name="gp", bufs=1) as gp,
            tc.tile_pool(name="yp", bufs=4) as yp,
            tc.tile_pool(name="psum", bufs=8, space="PSUM") as psum,
        ):
            w1_sb = {
                s: const.tile([P, MH, C], bf16, tag=f"w1{s}", name=f"w1{s}")
                for s in "AB"
            }
            w2_sb = {
                s: const.tile([P, MH, C], bf16, tag=f"w2{s}", name=f"w2{s}")
                for s in "AB"
            }
            b1_sb = {
                s: const.tile([P, MH], f32, tag=f"b1{s}", name=f"b1{s}")
                for s in "AB"
            }

            # --- initial loads, spread across queues ---
            # sync: x0 first half; scalar: x0 second half + b1A;
            # gpsimd: the whole segment-A weight stream in consumption order.
            x_tiles = {}
            T0 = tiles["A"][0]
            x_tiles[0] = xp.tile([P, KO1, T_TILE], bf16, tag="x", name="x0")
            for ko in range(KO1):
                eng = nc.sync if ko < 4 else nc.scalar
                eng.dma_start(x_tiles[0][:, ko : ko + 1, :T0], xr["A"][:, ko : ko + 1, :T0])
            nc.scalar.dma_start(b1_sb["A"][:], dram["b1A"][:])
            for m in range(MH):
                nc.gpsimd.dma_start(
                    w1_sb["A"][:, m : m + 1, :], dram["w1tA"][:, m : m + 1, :]
                )
            for ho in range(MH):
                nc.gpsimd.dma_start(
                    w2_sb["A"][:, ho : ho + 1, :], dram["w2tA"][:, ho : ho + 1, :]
                )

            for gi, (s, ti, T, t0) in enumerate(sched):
                # prefetch next tile's x (paced by the xp pool)
                if gi + 1 < len(sched):
                    ns, nti, nT, nt0 = sched[gi + 1]
                    x_tiles[gi + 1] = xp.tile(
                        [P, KO1, T_TILE], bf16, tag="x", name=f"x{gi + 1}"
                    )
                    nc.sync.dma_start(
                        x_tiles[gi + 1][:, :, :nT], xr[ns][:, :, nt0 : nt0 + nT]
                    )
                if gi == 1:
                    # segment-B constants stream while segment A computes
                    nc.gpsimd.dma_start(b1_sb["B"][:], dram["b1B"][:])
                    for m in range(MH):
                        nc.gpsimd.dma_start(
                            w1_sb["B"][:, m : m + 1, :], dram["w1tB"][:, m : m + 1, :]
                        )
                    for ho in range(MH):
                        nc.gpsimd.dma_start(
                            w2_sb["B"][:, ho : ho + 1, :], dram["w2tB"][:, ho : ho + 1, :]
                        )

                x_sb = x_tiles.pop(gi)
                g_sb = gp.tile([P, MH, T_TILE], bf16, tag="g", name=f"g{gi}")
                for m in range(MH):
                    ph = psum.tile([P, T_TILE], f32, tag="ps", name=f"ph{gi}_{m}")
                    for ko in range(KO1):
                        nc.tensor.matmul(
                            ph[:, :T],
                            w1_sb[s][:, m, ko * P : (ko + 1) * P],
                            x_sb[:, ko, :T],
                            start=(ko == 0),
                            stop=(ko == KO1 - 1),
                        )
                    nc.scalar.activation(
                        g_sb[:, m, :T], ph[:, :T], gelu, bias=b1_sb[s][:, m : m + 1]
                    )
                # layer 2, contraction split in two: the first half only needs
                # g[:8], so it issues right behind the L1 matmuls while the
                # m=15 gelu drains; the second half lands long after.
                pys = [
                    psum.tile([P, T_TILE], f32, tag="ps", name=f"py{gi}_{co}")
                    for co in range(KO1)
                ]
                for co in range(KO1):
                    for ho in range(MH // 2):
                        nc.tensor.matmul(
                            pys[co][:, :T],
                            w2_sb[s][:, ho, co * P : (co + 1) * P],
                            g_sb[:, ho, :T],
                            start=(ho == 0),
                            stop=False,
                        )
                for co in range(KO1):
                    for ho in range(MH // 2, MH):
                        nc.tensor.matmul(
                            pys[co][:, :T],
                            w2_sb[s][:, ho, co * P : (co + 1) * P],
                            g_sb[:, ho, :T],
                            start=False,
                            stop=(ho == MH - 1),
                        )
                    y_sb = yp.tile([P, T_TILE], bf16, tag="y", name=f"y{gi}_{co}")
                    nc.vector.tensor_copy(y_sb[:, :T], pys[co][:, :T])
                    # alternate store queues so the tail drains on two rings
                    eng = nc.sync if co % 2 == 0 else nc.gpsimd
                    eng.dma_start(yr[s][:, co, t0 : t0 + T], y_sb[:, :T])
    nc.finalize()
    return nc


def _route(flat_f32: np.ndarray, gate_w: np.ndarray):
    """Router, bit-matching the reference's jax ops (same env/backend)."""
    import jax
    import jax.numpy as jnp

    logits = jnp.asarray(flat_f32) @ jnp.asarray(gate_w).T
    probs = jax.nn.softmax(logits, axis=-1)
    top_p, top_i = jax.lax.top_k(probs, TOP_K)
    weights = top_p / (jnp.sum(top_p, axis=-1, keepdims=True) + 1e-8)
    return np.asarray(top_i), np.asarray(weights)


# results of the last device run, for test harness introspection
last_result = None


def _ensure_ntff_hook():
    """bass_utils' trace path imports antenv.axon_hooks, which the agent
    image's antenv lacks. Build the hook from trn_agent_boot's ctypes
    shim and inject a stand-in module."""
    import sys
    import types

    if "antenv.axon_hooks" in sys.modules:
        return
    try:
        from trn_agent_boot.trn_boot import _ntff_profile_via_ctypes

        hook = _ntff_profile_via_ctypes("/opt/axon/libaxon_pjrt.so")
    except Exception:
        hook = None
    m = types.ModuleType("antenv.axon_hooks")
    m.get_axon_ntff_profile_hook = lambda: hook
    m.set_axon_ntff_profile_hook = lambda h: None
    sys.modules["antenv.axon_hooks"] = m


def _prep_w1(w1e_half: np.ndarray) -> np.ndarray:
    # [HH, C] -> [ki, m, ko*128+j] with value w1[m*128+j, ko*128+ki]
    bf16 = ml_dtypes.bfloat16
    a = w1e_half.reshape(MH, P, KO1, P)          # [m, j, ko, ki]
    a = a.transpose(3, 0, 2, 1).reshape(P, MH, C)  # [ki, m, (ko j)]
    return np.ascontiguousarray(a).astype(bf16)


def _prep_w2(w2e_half: np.ndarray) -> np.ndarray:
    # [C, HH] -> [ki, ho, co*128+j] with value w2[co*128+j, ho*128+ki]
    bf16 = ml_dtypes.bfloat16
    a = w2e_half.reshape(KO1, P, MH, P)          # [co, j, ho, ki]
    a = a.transpose(3, 2, 0, 1).reshape(P, MH, C)  # [ki, ho, (co j)]
    return np.ascontiguousarray(a).astype(bf16)


def kernel(x, gate_w, w1, b1, w2, b2):
    from concourse.bass_utils import run_bass_kernel_spmd

    x = np.asarray(x)
    B, N, _ = x.shape
    flat = np.ascontiguousarray(x.reshape(-1, C), dtype=np.float32)
    w1 = np.asarray(w1, dtype=np.float32)
    w2 = np.asarray(w2, dtype=np.float32)
    b1 = np.asarray(b1, dtype=np.float32)
    b2 = np.asarray(b2, dtype=np.float32)

    top_i, weights = _route(flat, np.asarray(gate_w, dtype=np.float32))

    # token ids and combine weights per expert
    idx_e, g_e = [], []
    for e in range(N_EXPERTS):
        rows, cols = np.nonzero(top_i == e)
        idx_e.append(rows)
        g_e.append(weights[rows, cols].astype(np.float32))
    counts = np.array([len(i) for i in idx_e])

    # rank experts by load; segment A = big four, B = small four. Expert
    # ranked[r] runs as two hidden-halves on cores r and r+4; expert
    # ranked[7-r] likewise (segment B on the same core pair).
    ranked = np.argsort(-counts, kind="stable")
    pad = lambda n: max(int(-(-n // P) * P), P)
    capA = pad(int(counts[ranked[0]]))
    capB = pad(int(counts[ranked[4]]))

    key = (capA, capB)
    nc = _nc_cache.get(key)
    if nc is None:
        nc = _build_nc(capA, capB)
        _nc_cache[key] = nc

    bf16 = ml_dtypes.bfloat16

    # per-expert padded x (shared by the expert's two half-cores)
    xt = {}
    for s, cap, exps in (("A", capA, ranked[:4]), ("B", capB, ranked[4:])):
        for e in exps:
            xe = np.zeros((C, cap), dtype=bf16)
            xe[:, : counts[e]] = flat[idx_e[e]].T.astype(bf16)
            xt[int(e)] = xe

    in_maps = []
    for core in range(8):
        r, half = core % 4, core // 4
        m = {}
        for s, r_e in (("A", ranked[r]), ("B", ranked[7 - r])):
            e = int(r_e)
            lo, hi = half * HH, (half + 1) * HH
            m[f"xt{s}"] = xt[e]
            m[f"w1t{s}"] = _prep_w1(w1[e, lo:hi, :])
            m[f"w2t{s}"] = _prep_w2(w2[e, :, lo:hi])
            m[f"b1{s}"] = np.ascontiguousarray(b1[e, lo:hi].reshape(MH, P).T)
        in_maps.append(m)

    import os

    trace = bool(int(os.environ.get("MOE_TRACE", "0")))
    if trace:
        _ensure_ntff_hook()

    global last_result
    res = run_bass_kernel_spmd(
        nc,
        in_maps,
        core_ids=list(range(8)),
        trace=trace,
    )
    last_result = res

    out = np.zeros((flat.shape[0], C), dtype=np.float32)
    for r in range(4):
        for s, r_e in (("A", ranked[r]), ("B", ranked[7 - r])):
            e = int(r_e)
            cnt = counts[e]
            y = res.results[r][f"yt{s}"][:, :cnt].astype(np.float32)
            y += res.results[r + 4][f"yt{s}"][:, :cnt].astype(np.float32)
            out[idx_e[e]] += g_e[e][:, None] * (y.T + b2[e])
    return out.reshape(B, N, C)
